# revision 7
# baseline (speedup 1.0000x reference)
"""DeepHisCoM Trainium2 kernel (nn_DeepHisCoM_7017976562218).

Math (reference):
    xr = x.reshape(B, P, V)
    z1 = einsum('bpv,pwv->bpw', xr, W1)
    p  = leaky(einsum('bpw,pw->bp', leaky(z1), W2))
    BN(batch stats) -> global L2 normalize -> sigmoid(pn @ Wd + bd)

Device strategy (8 NeuronCores, PATHWAY-sharded: 16 pathways/core, full batch):
    - Host pre-transposes x to [P*V, B] fp8 (e3m4) so V lands on partitions
      directly from DMA — no on-chip transposes.
    - Per pathway: 128 matmuls, lhsT = x block [V=128, 128 batch] (stationary,
      fp8 FWL), rhs = W1^T_p [V, 64] fp8 -> z1 tile [128 b, 16 segs x 64]
      in PSUM (segments pack banks exactly: 8 x 64 fp32 per bank).
    - Layer 2 (sum_w W2_w * leaky(z1_w)) has two per-pathway pipelines to
      balance DVE vs ScalarE vs GpSimd:
      * scan pathway: fused DVE scan (running sum of w2 * leaky(z1)) per
        PSUM tile; per-(b, seg) sums = boundary-column diffs.
      * tree pathway: |w2| folded into the matmul weights, columns sorted
        positive-w2 first.  ScalarE Lrelu copies PSUM->SBUF bf16 in
        (w, seg) order applying sign*leaky exactly (pos: scale=1,
        alpha=0.2 -> leaky(y); neg: scale=-0.2, alpha=5 -> -leaky(y)).
        A halving add-tree (DVE level 1, GpSimd the rest) leaves the 16
        seg sums, written straight into the output tile.
      The pos/neg split is a compile-time AP shape, and the program is
      SPMD across cores, so tree "slots" are formed from groups of 8
      pathways with identical positive-count (the pathway->core
      assignment is free; the host permutes results back).
    - BN + global L2 + final linear + sigmoid on host (tiny).

Scale freedom: BN normalizes per pathway, so any per-pathway uniform scaling
of z2 cancels exactly; scan weights x32, tree weights x512 (|w2|-folded) to
sit in fp8 e3m4's normal range.
"""

import os
import sys

import numpy as np

for _p in ("/opt/trn_rl_repo",):
    if _p not in sys.path and os.path.isdir(_p):
        sys.path.insert(0, _p)

import ml_dtypes

import concourse.bacc as bacc
import concourse.bass as bass
import concourse.mybir as mybir
from concourse import dve_ops
from concourse.bass_utils import run_bass_kernel_spmd
from concourse.dve_spec import C0, AluOp, Spec, Src0, Src1, Zero, lower, maxx, scan
from concourse.dve_uop import DveOpSpec
from concourse.tile import TileContext


def _register_op(name, body, ref):
    """Register a custom DVE op, computing the uops shas in-container."""
    for op in dve_ops.OPS:
        if op.name == name:
            return op
    op = dve_ops.DveOp(name, Spec(body=body, reference=ref), subdim=False,
                       uops_sha={})
    dve_ops.OPS.append(op)
    dve_ops._SUB_OPCODE_FOR_NAME[name] = (
        dve_ops._CUSTOM_DVE_ROW_BASE + len(dve_ops.OPS) - 1
    )
    dve_ops.CUSTOM_DVE_SPECS[name] = op.spec
    opcode = dve_ops.get_dve_sub_opcode(name)
    for ver in ("v3", "v4"):
        spec_c = DveOpSpec(name=name, opcode=opcode,
                           uops=lower(op.spec, ver=ver),
                           rd1_en=True)
        op.uops_sha[ver] = spec_c.sha(ver)
    return op


def _leaky_scan_ref(in0, in1, s0, s1, imm2):
    i0 = in0.astype(np.float32).reshape(in0.shape[0], -1)
    i1 = in1.astype(np.float32).reshape(in1.shape[0], -1)
    prod = i0 * np.maximum(i1, s0 * i1)
    return np.cumsum(prod, axis=-1).reshape(in1.shape)


# out[t] = cumsum of in0[t] * leaky(in1[t]);  leaky(z) = max(z, s0*z)
LEAKY_SCAN_OP = _register_op(
    "STT_LEAKY_PREFIX_ANT",
    scan(AluOp.ADD, Src0 * maxx(Src1, Src1 * C0), init=Zero),
    _leaky_scan_ref,
)

P, V, W = 128, 128, 64
B = 16384
N_CORES = 8
PPC = P // N_CORES          # 16 pathways per core
NBT = B // 128              # 128 batch tiles (full batch per core)
SEGS = 16                   # segments (batch tiles) per PSUM tile
NTILE = NBT // SEGS         # 8 PSUM tiles per pathway
BN_EPS = 1e-5
SCAN_SCALE = 32.0           # per-pathway scales cancel in BN
TREE_SCALE = 512.0
LRELU = mybir.ActivationFunctionType.Lrelu
F32 = mybir.dt.float32
BF16 = mybir.dt.bfloat16
FP8 = mybir.dt.float8e3

_CACHE = {}
LAST_RESULTS = None


def _plan_slots(W2):
    """Greedy: tree slots = groups of 8 pathways with equal npos; leftovers
    scan.  Returns (slot_kinds, slot_npos, assign[16][8] global pathway ids),
    with tree/scan slots interleaved for engine balance."""
    npos = (W2 > 0).sum(axis=1)
    by = {}
    for p in range(P):
        by.setdefault(int(npos[p]), []).append(p)
    tree_groups = []
    scan_rest = []
    for v in sorted(by):
        lst = by[v]
        while len(lst) >= 8 and len(tree_groups) < PPC - 2:
            tree_groups.append((v, lst[:8]))
            lst = lst[8:]
        scan_rest.extend(lst)
    n_tree = len(tree_groups)
    n_scan = PPC - n_tree
    assert len(scan_rest) == n_scan * 8
    # interleave: roughly one scan per two trees
    kinds = []
    t_left, s_left = n_tree, n_scan
    while len(kinds) < PPC:
        if s_left > 0:
            kinds.append("S"); s_left -= 1
        for _ in range(2):
            if t_left > 0:
                kinds.append("T"); t_left -= 1
    slot_npos, assign = [], []
    ti = si = 0
    for k in kinds:
        if k == "T":
            v, grp = tree_groups[ti]; ti += 1
            slot_npos.append(v)
            assign.append(grp)
        else:
            slot_npos.append(-1)
            assign.append(scan_rest[si * 8:(si + 1) * 8])
            si += 1
    return kinds, slot_npos, assign


def _build_program(kinds, slot_npos, n_scan):
    nc = bacc.Bacc()
    x_in = nc.declare_dram_parameter("xs", [PPC * V, B], FP8, isOutput=False)
    wext_in = nc.declare_dram_parameter("wext", [V, PPC * W], FP8, isOutput=False)
    w2e_in = nc.declare_dram_parameter("w2e", [128, max(n_scan, 1) * SEGS * W],
                                       BF16, isOutput=False)
    ps_out = nc.declare_dram_parameter("ps", [128, PPC * NBT], F32, isOutput=True)

    with TileContext(nc) as tc:
        with (
            tc.tile_pool(name="singles", bufs=1) as singles,
            tc.tile_pool(name="xp", bufs=3) as xpool,
            tc.tile_pool(name="prod", bufs=3) as prodp,
            tc.tile_pool(name="s", bufs=3) as spool,
            tc.tile_pool(name="tr", bufs=3) as trpool,
            tc.tile_pool(name="hp", bufs=3, space="PSUM") as hpool,
        ):
            wext = singles.tile([V, PPC * W], FP8)
            nc.gpsimd.dma_start(out=wext[:], in_=wext_in[:, :])
            w2e = singles.tile([128, max(n_scan, 1) * SEGS * W], BF16)
            nc.gpsimd.dma_start(out=w2e[:], in_=w2e_in[:, :])
            p_sb = singles.tile([128, PPC * NBT], F32)

            rings = [nc.sync, nc.scalar, nc.gpsimd]
            ring_i = 0
            scan_idx = 0
            for pl in range(PPC):
                is_scan = kinds[pl] == "S"
                xp = xpool.tile([V, B], FP8, tag="x")
                for ch in range(2):   # 2 x 1 MiB chunks, rotating DMA rings
                    q = rings[ring_i % 3]; ring_i += 1
                    q.dma_start(
                        out=xp[:, ch * 8192:(ch + 1) * 8192],
                        in_=x_in[pl * V:(pl + 1) * V, ch * 8192:(ch + 1) * 8192],
                    )
                for t in range(NTILE):
                    hp = hpool.tile([128, SEGS * W], F32)
                    for j in range(SEGS):
                        bt = t * SEGS + j
                        nc.tensor.matmul(
                            hp[:, j * W:(j + 1) * W],
                            lhsT=xp[:, bt * 128:(bt + 1) * 128],
                            rhs=wext[:, pl * W:(pl + 1) * W],
                            start=True, stop=True,
                        )
                    base = pl * NBT + t * SEGS
                    if is_scan:
                        prod = prodp.tile([128, SEGS * W + 1], F32)
                        nc.gpsimd.memset(prod[:, 0:1], 0.0)
                        nc.vector._custom_dve(
                            LEAKY_SCAN_OP,
                            out=prod[:, 1:SEGS * W + 1].rearrange(
                                "p (s c) -> p s c", c=W),
                            in0=w2e[:, scan_idx * SEGS * W:
                                    (scan_idx + 1) * SEGS * W].rearrange(
                                "p (s c) -> p s c", c=W),
                            in1=hp[:].rearrange("p (s c) -> p s c", c=W),
                            s0=0.2,
                        )
                        hi = prod[:, 1:SEGS * W + 1].rearrange(
                            "p (s c) -> p s c", c=W)[:, :, W - 1:W].rearrange(
                            "p s c -> p (s c)")
                        lo = prod[:, 0:SEGS * W].rearrange(
                            "p (s c) -> p s c", c=W)[:, :, 0:1].rearrange(
                            "p s c -> p (s c)")
                        nc.vector.tensor_sub(
                            out=p_sb[:, base:base + SEGS], in0=hi, in1=lo)
                    else:
                        npp = slot_npos[pl]
                        # s[p, w, seg] = sign(w2) * leaky(|w2| z1) via Lrelu
                        sv = spool.tile([128, W * SEGS], BF16)
                        hv = hp[:].rearrange("p (s c) -> p c s", c=W)
                        if npp > 0:
                            nc.scalar.activation(
                                out=sv[:, :npp * SEGS].rearrange(
                                    "p (c s) -> p c s", s=SEGS),
                                in_=hv[:, 0:npp, :],
                                func=LRELU, scale=1.0, alpha=0.2,
                            )
                        if npp < W:
                            nc.scalar.activation(
                                out=sv[:, npp * SEGS:].rearrange(
                                    "p (c s) -> p c s", s=SEGS),
                                in_=hv[:, npp:W, :],
                                func=LRELU, scale=-0.2, alpha=5.0,
                            )
                        # halving add-tree over w: 1024 -> 16 seg sums
                        t1 = trpool.tile([128, W * SEGS // 2], BF16, tag="t1")
                        nc.vector.tensor_add(
                            out=t1[:], in0=sv[:, :W * SEGS // 2],
                            in1=sv[:, W * SEGS // 2:])
                        cur, n = t1, W * SEGS // 2
                        while n > 2 * SEGS:
                            nxt = trpool.tile([128, n // 2], BF16,
                                              tag=f"t{n // 2}")
                            nc.gpsimd.tensor_add(
                                out=nxt[:], in0=cur[:, :n // 2],
                                in1=cur[:, n // 2:n])
                            cur, n = nxt, n // 2
                        nc.gpsimd.tensor_add(
                            out=p_sb[:, base:base + SEGS],
                            in0=cur[:, :SEGS], in1=cur[:, SEGS:2 * SEGS])
                if is_scan:
                    scan_idx += 1
            nc.gpsimd.dma_start(out=ps_out[:, :], in_=p_sb[:])
    nc.finalize()
    return nc


def _prep_weights(W1, W2, kinds, slot_npos, assign, n_scan):
    """Per-core wext [V, PPC*W] fp8 and w2e (scan slots only) bf16."""
    W1 = W1.astype(np.float32)
    W2 = W2.astype(np.float32)
    wext = np.zeros((N_CORES, V, PPC * W), np.float32)
    w2e = np.zeros((N_CORES, max(n_scan, 1) * SEGS * W), np.float32)
    for c in range(N_CORES):
        si = 0
        for pl in range(PPC):
            p = assign[pl][c]
            if kinds[pl] == "S":
                wext[c, :, pl * W:(pl + 1) * W] = SCAN_SCALE * W1[p].T
                w2e[c, si * SEGS * W:(si + 1) * SEGS * W] = np.tile(W2[p], SEGS)
                si += 1
            else:
                npp = slot_npos[pl]
                pos = np.where(W2[p] > 0)[0]
                neg = np.where(W2[p] <= 0)[0]
                assert len(pos) == npp
                order = np.concatenate([pos, neg])
                cols = (np.abs(W2[p])[order][None, :]
                        * W1[p].T[:, order])            # [V, W] sorted
                wext[c, :, pl * W:(pl + 1) * W] = TREE_SCALE * cols
    wext8 = wext.astype(ml_dtypes.float8_e3m4)
    w2eb = np.ascontiguousarray(
        np.broadcast_to(w2e[:, None, :], (N_CORES, 128, w2e.shape[1]))
    ).astype(ml_dtypes.bfloat16)
    return wext8, w2eb


def kernel(x, W1, W2, gamma, beta, Wd, bd):
    global LAST_RESULTS
    x = np.ascontiguousarray(np.asarray(x, dtype=np.float32))
    W1 = np.asarray(W1, np.float32)
    W2 = np.asarray(W2, np.float32)

    kinds, slot_npos, assign = _plan_slots(W2)
    n_scan = sum(1 for k in kinds if k == "S")
    key = (tuple(kinds), tuple(slot_npos))
    if _CACHE.get("key") != key:
        _CACHE["nc"] = _build_program(kinds, slot_npos, n_scan)
        _CACHE["key"] = key
    nc = _CACHE["nc"]

    wext8, w2eb = _prep_weights(W1, W2, kinds, slot_npos, assign, n_scan)

    # host pre-transpose: x [B, P*V] -> xT [P*V, B] fp8 e3m4
    x8 = x.astype(ml_dtypes.float8_e3m4)
    xT = np.ascontiguousarray(x8.view(np.uint8).T)   # [P*V, B] uint8 view
    in_maps = []
    for c in range(N_CORES):
        rows = np.concatenate(
            [xT[assign[pl][c] * V:(assign[pl][c] + 1) * V, :]
             for pl in range(PPC)], axis=0)
        in_maps.append({
            "xs": rows.view(ml_dtypes.float8_e3m4),
            "wext": wext8[c],
            "w2e": w2eb[c],
        })
    res = run_bass_kernel_spmd(nc, in_maps, list(range(N_CORES)))
    LAST_RESULTS = res

    # ps[c]: [128 lanes, PPC*NBT], col = pl*NBT + bt; b = bt*128 + lane
    pvals = np.empty((B, P), np.float64)
    for c in range(N_CORES):
        pc = res.results[c]["ps"].astype(np.float64)
        arr = pc.reshape(128, PPC, NBT)                # [lane, pl, bt]
        blk = arr.transpose(2, 0, 1).reshape(B, PPC)   # [b, pl]
        for pl in range(PPC):
            pvals[:, assign[pl][c]] = blk[:, pl]
    # final leaky + BN(batch stats) + global L2 + sigmoid on host
    pvals = np.where(pvals >= 0, pvals, 0.2 * pvals)
    mean = pvals.mean(axis=0)
    var = pvals.var(axis=0)
    pn = (pvals - mean) / np.sqrt(var + BN_EPS) * np.asarray(gamma, np.float64) \
        + np.asarray(beta, np.float64)
    pn = pn / np.linalg.norm(pn)
    out = 1.0 / (1.0 + np.exp(-(pn @ np.asarray(Wd, np.float64)
                                + np.asarray(bd, np.float64))))
    return out.astype(np.float32)


# revision 13
# speedup vs baseline: 1.0513x; 1.0513x over previous
"""DeepHisCoM Trainium2 kernel (nn_DeepHisCoM_7017976562218).

Math (reference):
    xr = x.reshape(B, P, V)
    z1 = einsum('bpv,pwv->bpw', xr, W1)
    p  = leaky(einsum('bpw,pw->bp', leaky(z1), W2))
    BN(batch stats) -> global L2 normalize -> sigmoid(pn @ Wd + bd)

Device strategy (8 NeuronCores, PATHWAY-sharded: 16 pathways/core, full batch):
    - Host pre-transposes x to [P*V, B] fp8 (e3m4): V lands on partitions
      straight from DMA — no on-chip transposes.
    - Two per-pathway pipelines share the same x tiles, balancing DVE vs
      TensorE vs ScalarE:
      * scan pathway (12/core): 128 matmuls, lhsT = x block [V, 128 b]
        (stationary, fp8 FWL), rhs = 32*W1^T_p [V, 64] -> z1 [128 b,
        16 segs x 64] in PSUM; one fused DVE scan per tile (running sum of
        w2 * leaky(z1)); seg sums = boundary diffs (GpSimd subtract).
      * matmul pathway (4/core, processed in pairs): lhsT = 512*|w2|-folded
        W1^T_p [V, 64] stationary, x streamed 512 cols/matmul; two pathways
        stack PSUM partitions [0:64) and [64:128).  ScalarE Lrelu
        (alpha=0.2) applies leaky exactly; layer 2 = matmul with a +-1
        sign operand (per-core DATA -> SPMD-clean), out [2, 512] fp32
        drained PSUM->DRAM on the idle sync ring.
    - BN + global L2 + final linear + sigmoid on host (tiny).

Scale freedom: BN normalizes per pathway, so per-pathway uniform scalings of
z2 cancel exactly; scan weights x32, pair weights x512 (|w2|-folded) sit in
fp8 e3m4's normal range.
"""

import os
import sys

import numpy as np

for _p in ("/opt/trn_rl_repo",):
    if _p not in sys.path and os.path.isdir(_p):
        sys.path.insert(0, _p)

import ml_dtypes

import concourse.bacc as bacc
import concourse.bass as bass
import concourse.mybir as mybir
from concourse import dve_ops
from concourse.bass_utils import run_bass_kernel_spmd
from concourse.dve_spec import C0, AluOp, Spec, Src0, Src1, Zero, lower, maxx, scan
from concourse.dve_uop import DveOpSpec
from concourse.tile import TileContext


def _register_op(name, body, ref):
    """Register a custom DVE op, computing the uops shas in-container."""
    for op in dve_ops.OPS:
        if op.name == name:
            return op
    op = dve_ops.DveOp(name, Spec(body=body, reference=ref), subdim=False,
                       uops_sha={})
    dve_ops.OPS.append(op)
    dve_ops._SUB_OPCODE_FOR_NAME[name] = (
        dve_ops._CUSTOM_DVE_ROW_BASE + len(dve_ops.OPS) - 1
    )
    dve_ops.CUSTOM_DVE_SPECS[name] = op.spec
    opcode = dve_ops.get_dve_sub_opcode(name)
    for ver in ("v3", "v4"):
        spec_c = DveOpSpec(name=name, opcode=opcode,
                           uops=lower(op.spec, ver=ver),
                           rd1_en=True)
        op.uops_sha[ver] = spec_c.sha(ver)
    return op


def _leaky_scan_ref(in0, in1, s0, s1, imm2):
    i0 = in0.astype(np.float32).reshape(in0.shape[0], -1)
    i1 = in1.astype(np.float32).reshape(in1.shape[0], -1)
    prod = i0 * np.maximum(i1, s0 * i1)
    return np.cumsum(prod, axis=-1).reshape(in1.shape)


# out[t] = cumsum of in0[t] * leaky(in1[t]);  leaky(z) = max(z, s0*z)
LEAKY_SCAN_OP = _register_op(
    "STT_LEAKY_PREFIX_ANT",
    scan(AluOp.ADD, Src0 * maxx(Src1, Src1 * C0), init=Zero),
    _leaky_scan_ref,
)

P, V, W = 128, 128, 64
B = 16384
N_CORES = 8
PPC = P // N_CORES          # 16 pathways per core
NBT = B // 128              # 128 batch tiles (full batch per core)
SEGS = 16                   # segments per scan PSUM tile
NTILE = NBT // SEGS         # 8 scan PSUM tiles per pathway
NCH = B // 512              # 32 chunks per matmul-pathway pair
BN_EPS = 1e-5
SCAN_SCALE = 32.0           # per-pathway scales cancel in BN
PAIR_SCALE = 512.0
N_PAIRS = 2                 # matmul pathways = 2*N_PAIRS, rest scan
N_SCAN = PPC - 2 * N_PAIRS
LRELU = mybir.ActivationFunctionType.Lrelu
F32 = mybir.dt.float32
BF16 = mybir.dt.bfloat16
FP8 = mybir.dt.float8e3

# slot order: pathway slots 0..15; pairs placed mid-stream so engine load
# interleaves: S S S (B B) S S S (B B) S S S S S S
ORDER = ["S"] * 3 + ["B", "B"] + ["S"] * 3 + ["B", "B"] + ["S"] * 6
assert len(ORDER) == PPC and ORDER.count("B") == 2 * N_PAIRS

_CACHE = {}
LAST_RESULTS = None


def _build_program():
    nc = bacc.Bacc()
    x_in = nc.declare_dram_parameter("xs", [PPC * V, B], FP8, isOutput=False)
    wext_in = nc.declare_dram_parameter("wext", [V, PPC * W], FP8, isOutput=False)
    w2e_in = nc.declare_dram_parameter("w2e", [128, N_SCAN * SEGS * W], BF16,
                                       isOutput=False)
    # sgn[:, (pair, slot)]: [128, 32] sign matrix, nonzero only in cols
    # 2*slot, 2*slot+1 — 16 chunks accumulate into one [32, 512] PSUM tile
    sgn_in = nc.declare_dram_parameter("sgn", [128, N_PAIRS * 16 * 32], BF16,
                                       isOutput=False)
    ps_out = nc.declare_dram_parameter("ps", [128, N_SCAN * NBT], F32,
                                       isOutput=True)
    # zb[pair, half, 2*slot+member, n]; b = (16*half + slot)*512 + n
    zb_out = nc.declare_dram_parameter("zb", [N_PAIRS * 2 * 32, 512], F32,
                                       isOutput=True)

    with TileContext(nc) as tc:
        with (
            tc.tile_pool(name="singles", bufs=1) as singles,
            tc.tile_pool(name="xp", bufs=4) as xpool,
            tc.tile_pool(name="prod", bufs=3) as prodp,
            tc.tile_pool(name="sv", bufs=3) as spool,
            tc.tile_pool(name="hp", bufs=2, space="PSUM") as hpool,
            tc.tile_pool(name="zp", bufs=2, space="PSUM") as zpool,
            tc.tile_pool(name="z2", bufs=2, space="PSUM") as z2pool,
        ):
            wext = singles.tile([V, PPC * W], FP8)
            nc.gpsimd.dma_start(out=wext[:], in_=wext_in[:, :])
            w2e = singles.tile([128, N_SCAN * SEGS * W], BF16)
            nc.gpsimd.dma_start(out=w2e[:], in_=w2e_in[:, :])
            sgn = singles.tile([128, N_PAIRS * 16 * 32], BF16)
            nc.gpsimd.dma_start(out=sgn[:], in_=sgn_in[:, :])
            p_sb = singles.tile([128, N_SCAN * NBT], F32)

            rings = [nc.sync, nc.scalar, nc.gpsimd]
            ring_i = 0

            def load_x(pl):
                xp = xpool.tile([V, B], FP8, tag="x")
                nonlocal ring_i
                for ch in range(2):
                    q = rings[ring_i % 3]; ring_i += 1
                    q.dma_start(
                        out=xp[:, ch * 8192:(ch + 1) * 8192],
                        in_=x_in[pl * V:(pl + 1) * V, ch * 8192:(ch + 1) * 8192],
                    )
                return xp

            def scan_pathway(pl, si, xp):
                for t in range(NTILE):
                    hp = hpool.tile([128, SEGS * W], F32)
                    for j in range(SEGS):
                        bt = t * SEGS + j
                        nc.tensor.matmul(
                            hp[:, j * W:(j + 1) * W],
                            lhsT=xp[:, bt * 128:(bt + 1) * 128],
                            rhs=wext[:, pl * W:(pl + 1) * W],
                            start=True, stop=True,
                        )
                    prod = prodp.tile([128, SEGS * W + 1], F32)
                    nc.gpsimd.memset(prod[:, 0:1], 0.0)
                    nc.vector._custom_dve(
                        LEAKY_SCAN_OP,
                        out=prod[:, 1:SEGS * W + 1].rearrange(
                            "p (s c) -> p s c", c=W),
                        in0=w2e[:, si * SEGS * W:(si + 1) * SEGS * W].rearrange(
                            "p (s c) -> p s c", c=W),
                        in1=hp[:].rearrange("p (s c) -> p s c", c=W),
                        s0=0.2,
                    )
                    hi = prod[:, 1:SEGS * W + 1].rearrange(
                        "p (s c) -> p s c", c=W)[:, :, W - 1:W].rearrange(
                        "p s c -> p (s c)")
                    lo = prod[:, 0:SEGS * W].rearrange(
                        "p (s c) -> p s c", c=W)[:, :, 0:1].rearrange(
                        "p s c -> p (s c)")
                    base = si * NBT + t * SEGS
                    nc.gpsimd.tensor_sub(
                        out=p_sb[:, base:base + SEGS], in0=hi, in1=lo)

            def pair_pathways(pla, plb, pair, xpa, xpb):
                z2 = None
                for c in range(NCH):
                    zp = zpool.tile([128, 512], F32)
                    nc.tensor.matmul(
                        zp[0:64, :],
                        lhsT=wext[:, pla * W:(pla + 1) * W],
                        rhs=xpa[:, c * 512:(c + 1) * 512],
                        start=True, stop=True,
                    )
                    nc.tensor.matmul(
                        zp[64:128, :],
                        lhsT=wext[:, plb * W:(plb + 1) * W],
                        rhs=xpb[:, c * 512:(c + 1) * 512],
                        start=True, stop=True,
                    )
                    sv = spool.tile([128, 512], BF16)
                    nc.scalar.activation(out=sv[:], in_=zp[:],
                                         func=LRELU, scale=1.0, alpha=0.2)
                    slot = c % 16
                    if slot == 0:
                        z2 = z2pool.tile([32, 512], F32)
                    nc.tensor.matmul(
                        z2[:],
                        lhsT=sgn[:, (pair * 16 + slot) * 32:
                                 (pair * 16 + slot + 1) * 32],
                        rhs=sv[:],
                        start=(slot == 0), stop=(slot == 15),
                    )
                    if slot == 15:
                        half = c // 16
                        z2s = spool.tile([32, 512], F32, tag="z2s")
                        nc.scalar.copy(out=z2s[:], in_=z2[:])
                        nc.sync.dma_start(
                            out=zb_out[(pair * 2 + half) * 32:
                                       (pair * 2 + half + 1) * 32, :],
                            in_=z2s[:],
                        )

            si = 0
            pair = 0
            pl = 0
            while pl < PPC:
                if ORDER[pl] == "S":
                    xp = load_x(pl)
                    scan_pathway(pl, si, xp)
                    si += 1
                    pl += 1
                else:
                    xpa = load_x(pl)
                    xpb = load_x(pl + 1)
                    pair_pathways(pl, pl + 1, pair, xpa, xpb)
                    pair += 1
                    pl += 2
            nc.gpsimd.dma_start(out=ps_out[:, :], in_=p_sb[:])
    nc.finalize()
    return nc


def _prep_weights(W1, W2, assign):
    """Per-core wext [V, PPC*W] fp8, w2e (scan slots) bf16, sgn [128, 2*pairs]."""
    W1 = W1.astype(np.float32)
    W2 = W2.astype(np.float32)
    wext = np.zeros((N_CORES, V, PPC * W), np.float32)
    w2e = np.zeros((N_CORES, N_SCAN * SEGS * W), np.float32)
    sgn = np.zeros((N_CORES, 128, N_PAIRS * 16 * 32), np.float32)
    for c in range(N_CORES):
        si = 0
        pair = 0
        pl = 0
        while pl < PPC:
            p = assign[pl][c]
            if ORDER[pl] == "S":
                wext[c, :, pl * W:(pl + 1) * W] = SCAN_SCALE * W1[p].T
                w2e[c, si * SEGS * W:(si + 1) * SEGS * W] = np.tile(W2[p], SEGS)
                si += 1
                pl += 1
            else:
                pb = assign[pl + 1][c]
                wext[c, :, pl * W:(pl + 1) * W] = (
                    PAIR_SCALE * np.abs(W2[p])[None, :] * W1[p].T)
                wext[c, :, (pl + 1) * W:(pl + 2) * W] = (
                    PAIR_SCALE * np.abs(W2[pb])[None, :] * W1[pb].T)
                for slot in range(16):
                    base = (pair * 16 + slot) * 32
                    sgn[c, 0:64, base + 2 * slot] = np.sign(W2[p])
                    sgn[c, 64:128, base + 2 * slot + 1] = np.sign(W2[pb])
                pair += 1
                pl += 2
    return (wext.astype(ml_dtypes.float8_e3m4),
            np.ascontiguousarray(
                np.broadcast_to(w2e[:, None, :],
                                (N_CORES, 128, w2e.shape[1]))
            ).astype(ml_dtypes.bfloat16),
            sgn.astype(ml_dtypes.bfloat16))


def kernel(x, W1, W2, gamma, beta, Wd, bd):
    global LAST_RESULTS
    x = np.ascontiguousarray(np.asarray(x, dtype=np.float32))
    W1 = np.asarray(W1, np.float32)
    W2 = np.asarray(W2, np.float32)

    # pathway assignment: slot pl of core c gets global pathway c*PPC + pl
    assign = [[c * PPC + pl for c in range(N_CORES)] for pl in range(PPC)]

    if "nc" not in _CACHE:
        _CACHE["nc"] = _build_program()
    nc = _CACHE["nc"]

    wext8, w2eb, sgnb = _prep_weights(W1, W2, assign)

    # host pre-transpose: x [B, P*V] -> xT [P*V, B] fp8 e3m4
    x8 = x.astype(ml_dtypes.float8_e3m4)
    xT = np.ascontiguousarray(x8.view(np.uint8).T)
    in_maps = [
        {
            "xs": xT[c * PPC * V:(c + 1) * PPC * V, :].view(
                ml_dtypes.float8_e3m4),
            "wext": wext8[c],
            "w2e": w2eb[c],
            "sgn": sgnb[c],
        }
        for c in range(N_CORES)
    ]
    res = run_bass_kernel_spmd(nc, in_maps, list(range(N_CORES)))
    LAST_RESULTS = res

    pvals = np.empty((B, P), np.float64)
    for c in range(N_CORES):
        ps = res.results[c]["ps"].astype(np.float64)   # [128, N_SCAN*128]
        # zb rows (pair*2+half)*32 + 2*slot + member, cols n;
        # b = (16*half + slot)*512 + n
        zb = res.results[c]["zb"].astype(np.float64)
        zb = zb.reshape(N_PAIRS, 2, 16, 2, 512)        # [q, half, slot, m, n]
        zv = zb.transpose(0, 3, 1, 2, 4).reshape(N_PAIRS, 2, B)  # [q, m, b]
        si = 0
        pair = 0
        pl = 0
        while pl < PPC:
            p = assign[pl][c]
            if ORDER[pl] == "S":
                blk = ps[:, si * NBT:(si + 1) * NBT]   # [lane, bt]
                pvals[:, p] = blk.T.reshape(B)         # b = bt*128 + lane
                si += 1
                pl += 1
            else:
                pb = assign[pl + 1][c]
                pvals[:, p] = zv[pair, 0]
                pvals[:, pb] = zv[pair, 1]
                pair += 1
                pl += 2
    # final leaky + BN(batch stats) + global L2 + sigmoid on host
    pvals = np.where(pvals >= 0, pvals, 0.2 * pvals)
    mean = pvals.mean(axis=0)
    var = pvals.var(axis=0)
    pn = (pvals - mean) / np.sqrt(var + BN_EPS) * np.asarray(gamma, np.float64) \
        + np.asarray(beta, np.float64)
    pn = pn / np.linalg.norm(pn)
    out = 1.0 / (1.0 + np.exp(-(pn @ np.asarray(Wd, np.float64)
                                + np.asarray(bd, np.float64))))
    return out.astype(np.float32)
